# revision 1
# baseline (speedup 1.0000x reference)
"""Trainium2 Bass kernel for llama-style GQA attention block.

Problem (hardcoded): x[1,2048,2048] f32, 32 q heads / 8 kv heads, head_dim 64,
RoPE (interleaved pairs), causal mask, out proj. 8-core tensor parallel across
heads: each core owns 4 q heads + 1 kv head, computes its slice end-to-end
including its wo row-block partial product; host sums the 8 partials.

All matmuls run as float32r (fp32 data, fast PE mode, ~1e-4 rel err).
Layout is "feature-on-partition" (transposed) throughout so every matmul
contracts over the partition dim with no on-chip transposes of activations:
  QT = wq^T x^T       (via lhsT=wq, rhs=xT)
  ST = K Q^T          (via lhsT=KT, rhs=QT)   -> softmax along partitions
  OT = [V|1]^T PT     (via lhsT=Vext, rhs=PT) -> row 64 = softmax denominator
  out = OT^T wo       (via lhsT=OT, rhs=wo)
RoPE even/odd pairs are separated by a host-side wq/wk column permutation:
all 4 heads' even-index dims occupy partitions 0..127 of QeP (32 rows per
head), odds in QoP. The kv head's rotated K is replicated 4x along
partitions (KrepE/KrepO) so the 4 heads' score matmuls run as concurrent
32-row PE row-groups (tile_position 0/32/64/96), contracting K=32 each with
even+odd accumulated in PSUM.
"""

import numpy as np

import concourse.bass as bass
import concourse.bacc as bacc
import concourse.mybir as mybir
from concourse.tile import TileContext
from concourse.bass_utils import run_bass_kernel_spmd

DIM = 2048
SEQ = 2048
N_HEADS = 32
N_KV = 8
HD = 64
NCORES = 8
HPC = N_HEADS // NCORES      # 4 q heads per core
SC = 512                     # seq chunk (matmul free dim)
NSC = SEQ // SC              # 4
KTILE = 128
NKT = SEQ // KTILE           # 16
NDCH = DIM // 128            # 16 contraction chunks for projections
F32 = mybir.dt.float32
F32R = mybir.dt.float32r
NEG = -1.0e30

_CACHE = {}


def _build_nc(reps=1, reload_weights=True):
    nc = bacc.Bacc("TRN2", debug=False, num_devices=NCORES)
    xT_p = nc.declare_dram_parameter("xT", [DIM, SEQ], F32R, isOutput=False)
    wq_p = nc.declare_dram_parameter("wq", [DIM, HPC * HD], F32R, isOutput=False)
    wkv_p = nc.declare_dram_parameter("wkv", [DIM, 2 * HD], F32R, isOutput=False)
    wo_p = nc.declare_dram_parameter("wo", [HPC * HD, DIM], F32R, isOutput=False)
    cs_p = nc.declare_dram_parameter("cs", [256, SEQ], F32, isOutput=False)
    pat_p = nc.declare_dram_parameter("pat", [KTILE, KTILE], F32, isOutput=False)
    id_p = nc.declare_dram_parameter("ident", [HD, HD], F32, isOutput=False)
    out_p = nc.declare_dram_parameter("out", [SEQ, DIM], F32, isOutput=True)

    xT_r = xT_p.rearrange("(k p) s -> k p s", p=128)
    wq_r = wq_p.rearrange("(k p) m -> p k m", p=128)
    wkv_r = wkv_p.rearrange("(k p) m -> p k m", p=128)
    EXP = mybir.ActivationFunctionType.Exp

    with TileContext(nc) as tc:
        with (
            tc.tile_pool(name="res", bufs=1) as res,
            tc.tile_pool(name="sb", bufs=2) as sb,
            tc.tile_pool(name="psum", bufs=1, space="PSUM") as psum,
        ):
            ps_ctr = [0]

            def ps_tile(idx, shape=(128, SC)):
                ps_ctr[0] += 1
                return psum.tile(list(shape), F32, tag=f"p{idx}", name=f"ps{ps_ctr[0]}")

            # ---- resident allocs ----
            wq_t = res.tile([128, NDCH, HPC * HD], F32R, tag="wq_t")
            wkv_t = res.tile([128, NDCH, 2 * HD], F32R, tag="wkv_t")
            wo0_t = res.tile([128, DIM], F32R, tag="wo0_t")
            wo1_t = res.tile([128, DIM], F32R, tag="wo1_t")
            cos4 = res.tile([128, SEQ], F32, tag="cos4")
            sin4 = res.tile([128, SEQ], F32, tag="sin4")
            pat_t = res.tile([128, 128], F32, tag="pat_t")
            ident = res.tile([HD, HD], F32, tag="ident")

            # ---- resident intermediates ----
            QeP = res.tile([128, SEQ], F32R, tag="QeP")    # 4 heads x 32 even rows
            QoP = res.tile([128, SEQ], F32R, tag="QoP")
            KrepE = res.tile([128, SEQ], F32R, tag="KrepE")  # kv head x4 copies
            KrepO = res.tile([128, SEQ], F32R, tag="KrepO")
            VT_sb = res.tile([HD, SEQ], F32, tag="VT_sb")
            OTn0 = res.tile([128, SEQ], F32R, tag="OTn0")   # heads 0,1 norm out^T
            OTn1 = res.tile([128, SEQ], F32R, tag="OTn1")
            ones_col = res.tile([128, 1], F32, tag="ones_col")
            nc.vector.memset(ones_col[:], 1.0)
            ones_f32 = res.tile([1, HD], F32, tag="ones_f32")
            nc.vector.memset(ones_f32[:], 1.0)
            ones_row = res.tile([1, HD], F32R, tag="ones_row")
            nc.vector.tensor_copy(ones_row[:], ones_f32[:])
            vext = []
            for kt in range(NKT):
                vx = res.tile([128, HD + 1], F32R, tag=f"vx{kt}")
                nc.vector.tensor_copy(vx[:, HD : HD + 1], ones_col[:])
                vext.append(vx)

            for _rep in range(reps):
                for sc in range(NSC):
                    slc = slice(sc * SC, (sc + 1) * SC)
                    # ---- proj(sc): QKV projections (banks p0, p1, p2) ----
                    qe_ps = ps_tile(0)
                    qo_ps = ps_tile(1)
                    kv_ps = ps_tile(2)
                    for k in range(NDCH):
                        # stagger weight/constant loads into chunk 0's k-loop
                        if sc == 0 and (_rep == 0 or reload_weights):
                            nc.sync.dma_start(out=wq_t[:, k, :], in_=wq_r[:, k, :])
                            nc.sync.dma_start(out=wkv_t[:, k, :], in_=wkv_r[:, k, :])
                            if k == 1:
                                nc.sync.dma_start(out=cos4[:], in_=cs_p[0:128, :])
                                nc.sync.dma_start(out=sin4[:], in_=cs_p[128:256, :])
                            if k == 2:
                                nc.sync.dma_start(out=pat_t[:], in_=pat_p[:, :])
                                nc.sync.dma_start(out=ident[:], in_=id_p[:, :])
                            if k == 8:
                                nc.sync.dma_start(out=wo0_t[:], in_=wo_p[0:128, :])
                            if k == 12:
                                nc.sync.dma_start(out=wo1_t[:], in_=wo_p[128:256, :])
                        xt = sb.tile([128, SC], F32R, tag="xt", bufs=4)
                        nc.sync.dma_start(out=xt[:], in_=xT_r[k, :, slc])
                        st, sp = (k == 0), (k == NDCH - 1)
                        nc.tensor.matmul(qe_ps[:], wq_t[:, k, 0:128], xt[:], start=st, stop=sp)
                        nc.tensor.matmul(qo_ps[:], wq_t[:, k, 128:256], xt[:], start=st, stop=sp)
                        nc.tensor.matmul(kv_ps[:], wkv_t[:, k, :], xt[:], start=st, stop=sp)
                    # rope Q (full 128-lane)
                    t1 = sb.tile([128, SC], F32, tag="t1", bufs=2)
                    t2 = sb.tile([128, SC], F32, tag="t2", bufs=2)
                    nc.vector.tensor_mul(t1[:], qe_ps[:], cos4[:, slc])
                    nc.vector.tensor_mul(t2[:], qo_ps[:], sin4[:, slc])
                    nc.vector.tensor_sub(QeP[:, slc], t1[:], t2[:])
                    t3 = sb.tile([128, SC], F32, tag="t3", bufs=2)
                    t4 = sb.tile([128, SC], F32, tag="t4", bufs=2)
                    nc.vector.tensor_mul(t3[:], qo_ps[:], cos4[:, slc])
                    nc.vector.tensor_mul(t4[:], qe_ps[:], sin4[:, slc])
                    nc.vector.tensor_add(QoP[:, slc], t3[:], t4[:])
                    # rope K + replicate x4
                    k1 = sb.tile([32, SC], F32, tag="k1", bufs=2)
                    k2 = sb.tile([32, SC], F32, tag="k2", bufs=2)
                    nc.vector.tensor_mul(k1[:], kv_ps[0:32, :], cos4[0:32, slc])
                    nc.vector.tensor_mul(k2[:], kv_ps[32:64, :], sin4[0:32, slc])
                    nc.vector.tensor_sub(KrepE[0:32, slc], k1[:], k2[:])
                    k3 = sb.tile([32, SC], F32, tag="k3", bufs=2)
                    k4 = sb.tile([32, SC], F32, tag="k4", bufs=2)
                    nc.vector.tensor_mul(k3[:], kv_ps[32:64, :], cos4[0:32, slc])
                    nc.vector.tensor_mul(k4[:], kv_ps[0:32, :], sin4[0:32, slc])
                    nc.vector.tensor_add(KrepO[0:32, slc], k3[:], k4[:])
                    for r in (32, 64, 96):
                        nc.vector.tensor_copy(KrepE[r : r + 32, slc], KrepE[0:32, slc])
                        nc.vector.tensor_copy(KrepO[r : r + 32, slc], KrepO[0:32, slc])
                    # V passthrough
                    nc.vector.tensor_copy(VT_sb[:, slc], kv_ps[HD:128, :])

                    # ---- vtrans(sc): V transpose for this chunk (bank p7) ----
                    for kt in range(4 * sc, 4 * sc + 4):
                        vt_ps = ps_tile(7, (128, HD))
                        nc.tensor.transpose(vt_ps[:], VT_sb[:, kt * 128 : (kt + 1) * 128], ident[:])
                        nc.vector.tensor_copy(vext[kt][:, 0:HD], vt_ps[:])

                    # ---- attention(sc): head pairs, banks st p3/p4 ot p5/p6 ----
                    nkt_h = 4 * sc + 4
                    for hp in range(2):
                        heads = (2 * hp, 2 * hp + 1)
                        o_ps = {h: ps_tile(5 + i, (HD + 1, SC)) for i, h in enumerate(heads)}
                        for kt in range(nkt_h):
                            ksl = slice(kt * 128, (kt + 1) * 128)
                            j = kt - 4 * sc
                            # diagonal tiles: only q >= k-tile start contributes
                            qo = 128 * j if j > 0 else 0      # q offset within chunk
                            nv = SC - qo                      # valid q count
                            qsl = slice(sc * SC + qo, (sc + 1) * SC)
                            st_ps = {h: ps_tile(3 + i) for i, h in enumerate(heads)}
                            for h in heads:
                                rows = slice(32 * h, 32 * h + 32)
                                tp = (96, 0) if h == 3 else None
                                nc.tensor.matmul(st_ps[h][:, 0:nv], KrepE[rows, ksl], QeP[rows, qsl],
                                                 start=True, stop=False, tile_position=tp)
                                nc.tensor.matmul(st_ps[h][:, 0:nv], KrepO[rows, ksl], QoP[rows, qsl],
                                                 start=False, stop=True, tile_position=tp)
                            for h in heads:
                                if j >= 0:
                                    # triangle mask on the first 128 valid columns
                                    nc.vector.tensor_add(st_ps[h][:, 0:128], st_ps[h][:, 0:128], pat_t[:])
                                ptile = sb.tile([128, SC], F32R, tag="pt", bufs=6)
                                nc.scalar.activation(ptile[:, 0:nv], st_ps[h][:, 0:nv], EXP, scale=0.125)
                                nc.tensor.matmul(o_ps[h][:, qo : qo + nv], vext[kt][:], ptile[:, 0:nv],
                                                 start=(kt == 0), stop=(kt == nkt_h - 1))
                        for h in heads:
                            g, hh = h // 2, h % 2
                            recip = sb.tile([1, SC], F32R, tag="recip", bufs=2)
                            with nc.allow_low_precision(reason="f32r is fp32-width"):
                                nc.vector.reciprocal(recip[:], o_ps[h][HD : HD + 1, :])
                            bc_ps = ps_tile(3 + (h % 2), (HD, SC))
                            nc.tensor.matmul(bc_ps[:], ones_row[:], recip[:], start=True, stop=True)
                            ou_sb = sb.tile([HD, SC], F32, tag="ou", bufs=2)
                            nc.vector.tensor_copy(ou_sb[:], o_ps[h][0:HD, :])
                            dst = (OTn0, OTn1)[g]
                            nc.vector.tensor_mul(dst[64 * hh : 64 * hh + 64, slc], ou_sb[:], bc_ps[:])

                    # ---- outproj(sc): rows of this chunk (bank p7) ----
                    for st in range(4 * sc, 4 * sc + 4):
                        ssl = slice(st * 128, (st + 1) * 128)
                        for dch in range(NSC):
                            dsl = slice(dch * SC, (dch + 1) * SC)
                            op_ps = ps_tile(7)
                            nc.tensor.matmul(op_ps[:], OTn0[:, ssl], wo0_t[:, dsl], start=True, stop=False)
                            nc.tensor.matmul(op_ps[:], OTn1[:, ssl], wo1_t[:, dsl], start=False, stop=True)
                            ot = sb.tile([128, SC], F32, tag="ot", bufs=4)
                            nc.vector.tensor_copy(ot[:], op_ps[:])
                            nc.sync.dma_start(out=out_p[ssl, dsl], in_=ot[:])

    nc.compile()
    return nc


def _host_prep(x, freqs_cos, freqs_sin):
    """Shared (core-independent) host-side tensors."""
    xT = np.ascontiguousarray(np.asarray(x, np.float32)[0].T)          # [DIM, SEQ]
    cosT = np.ascontiguousarray(np.asarray(freqs_cos, np.float32).T)   # [32, SEQ]
    sinT = np.ascontiguousarray(np.asarray(freqs_sin, np.float32).T)
    cs = np.concatenate([np.tile(cosT, (4, 1)), np.tile(sinT, (4, 1))], 0)  # [256, SEQ]
    kk = np.arange(KTILE)[:, None]
    qq = np.arange(KTILE)[None, :]
    pat = np.where(kk <= qq, 0.0, NEG).astype(np.float32)              # [128, 128]
    return xT, cs, pat


def _perm_q():
    """wq columns -> [all heads' even dims (4x32), all heads' odd dims]."""
    ev = [h * HD + 2 * i for h in range(HPC) for i in range(HD // 2)]
    od = [h * HD + 2 * i + 1 for h in range(HPC) for i in range(HD // 2)]
    return ev + od


def _perm_k():
    """wk columns (single head) -> [even dims (32), odd dims (32)]."""
    return [2 * i for i in range(HD // 2)] + [2 * i + 1 for i in range(HD // 2)]


def _is_causal(mask):
    m = np.asarray(mask)
    if m.shape != (SEQ, SEQ):
        return False
    tril = np.tril(np.ones((SEQ, SEQ), bool))
    return bool(np.all(m[tril] == 0.0) and np.all(np.isneginf(m[~tril])))


def _numpy_fallback(x, freqs_cos, freqs_sin, mask, wq, wk, wv, wo):
    x = np.asarray(x, np.float64)
    b, s, _ = x.shape
    xq = (x @ wq).reshape(b, s, N_HEADS, HD)
    xk = (x @ wk).reshape(b, s, N_KV, HD)
    xv = (x @ wv).reshape(b, s, N_KV, HD)

    def rope(t):
        t2 = t.reshape(*t.shape[:-1], HD // 2, 2)
        te, to = t2[..., 0], t2[..., 1]
        c = np.asarray(freqs_cos, np.float64)[None, :, None, :]
        sn = np.asarray(freqs_sin, np.float64)[None, :, None, :]
        oe = te * c - to * sn
        oo = te * sn + to * c
        return np.stack([oe, oo], -1).reshape(t.shape)

    xq, xk = rope(xq), rope(xk)
    xk = np.repeat(xk, N_HEADS // N_KV, axis=2)
    xv = np.repeat(xv, N_HEADS // N_KV, axis=2)
    sc_ = np.einsum("bqhd,bkhd->bhqk", xq, xk) / np.sqrt(HD)
    sc_ = sc_ + np.asarray(mask, np.float64)[None, None]
    m = sc_.max(-1, keepdims=True)
    p = np.exp(sc_ - m)
    p = p / p.sum(-1, keepdims=True)
    out = np.einsum("bhqk,bkhd->bqhd", p, xv).reshape(b, s, N_HEADS * HD)
    return (out @ wo).astype(np.float32)


def _make_in_maps(x, freqs_cos, freqs_sin, wq, wk, wv, wo):
    xT, cs, pat = _host_prep(x, freqs_cos, freqs_sin)
    wq = np.asarray(wq, np.float32)
    wk = np.asarray(wk, np.float32)
    wv = np.asarray(wv, np.float32)
    wo = np.asarray(wo, np.float32)
    permq = _perm_q()
    permk = _perm_k()
    in_maps = []
    for c in range(NCORES):
        wq_c = np.ascontiguousarray(wq[:, c * 256 : (c + 1) * 256][:, permq])
        wk_c = wk[:, c * HD : (c + 1) * HD][:, permk]
        wv_c = wv[:, c * HD : (c + 1) * HD]
        wkv_c = np.ascontiguousarray(np.concatenate([wk_c, wv_c], 1))
        wo_c = np.ascontiguousarray(wo[c * 256 : (c + 1) * 256, :])
        im = {"xT": xT, "wq": wq_c, "wkv": wkv_c, "wo": wo_c, "cs": cs, "pat": pat,
              "ident": np.eye(HD, dtype=np.float32)}
        expect = {"xT": (DIM, SEQ), "wq": (DIM, HPC * HD), "wkv": (DIM, 2 * HD),
                  "wo": (HPC * HD, DIM), "cs": (256, SEQ), "pat": (KTILE, KTILE),
                  "ident": (HD, HD)}
        for k_, v_ in im.items():
            assert v_.shape == expect[k_], (k_, v_.shape, expect[k_])
        in_maps.append(im)
    return in_maps


def get_nc(reps=1, reload_weights=True):
    key = f"nc{reps}_{reload_weights}"
    if key not in _CACHE:
        _CACHE[key] = _build_nc(reps, reload_weights)
    return _CACHE[key]


def kernel(x, freqs_cos, freqs_sin, mask, wq, wk, wv, wo):
    if not _is_causal(mask):
        return _numpy_fallback(x, freqs_cos, freqs_sin, mask, wq, wk, wv, wo)
    nc = get_nc()
    in_maps = _make_in_maps(x, freqs_cos, freqs_sin, wq, wk, wv, wo)
    res = run_bass_kernel_spmd(nc, in_maps, list(range(NCORES))).results
    acc = res[0]["out"].astype(np.float64)
    for c in range(1, NCORES):
        acc += res[c]["out"]
    return acc.astype(np.float32)[None]



# revision 2
# speedup vs baseline: 1.1185x; 1.1185x over previous
"""Trainium2 Bass kernel v2 for llama-style GQA attention block.

Per-core (8-way tensor parallel over heads): 4 q heads + 1 kv head, fp16
matmul operands everywhere (fp32 PSUM accumulation), fp16 partial output;
host sums the 8 partials.

Structure per seq chunk sc (SC=512):
  proj:   QKV projections (fp16 x/w) into PSUM pair-tiles st01 (banks 0-1:
          qe, qo) and st23 (bank 2: kv), with half of the PREVIOUS chunk's
          out-projection tiles interleaved into the k-loop.
  rope:   DVE fp32 math out of PSUM, fp16 outputs; K replication on GpSimd.
  vtrans: PE transposes V^T -> V (fp16 PSUM bank 7), duplicated into
          vext2=[V|V].
  attn:   per k-tile: 8 score matmuls (4 heads x even/odd, K=32 row-tiled at
          row groups 0/32/64/96) into st01/st23, per-pair fused exp on ACT
          (strided AP across 2 PSUM banks) -> fp16 pt4, col-tiled PV pairs
          into shared o-banks (memset + start=False accumulation), col-tiled
          M=1 denominator matmuls into bank-7 rows 0/32/64/96. Remaining
          out-projection tiles of chunk sc-1 interleave here.
  norm:   denominators -> base-0 tiles -> reciprocal_approx_fast -> PE
          broadcast (col-tiled pairs into st01) -> one fused TT per o-bank.
"""

import numpy as np

import concourse.bass as bass
import concourse.bacc as bacc
import concourse.mybir as mybir
from concourse.tile import TileContext
from concourse.bass_utils import run_bass_kernel_spmd

DIM = 2048
SEQ = 2048
N_HEADS = 32
N_KV = 8
HD = 64
NCORES = 8
HPC = N_HEADS // NCORES      # 4 q heads per core
SC = 512                     # seq chunk (matmul free dim)
NSC = SEQ // SC              # 4
KTILE = 128
NKT = SEQ // KTILE           # 16
NDCH = DIM // 128            # 16 contraction chunks for projections
F32 = mybir.dt.float32
F32R = mybir.dt.float32r
F16 = mybir.dt.float16
NEG = -1.0e30

_CACHE = {}


def _build_nc(reps=1, reload_weights=True):
    nc = bacc.Bacc("TRN2", debug=False, num_devices=NCORES)
    xT_p = nc.declare_dram_parameter("xT", [DIM, SEQ], F16, isOutput=False)
    wq_p = nc.declare_dram_parameter("wq", [DIM, HPC * HD], F16, isOutput=False)
    wkv_p = nc.declare_dram_parameter("wkv", [DIM, 2 * HD], F16, isOutput=False)
    wo_p = nc.declare_dram_parameter("wo", [HPC * HD, DIM], F16, isOutput=False)
    cs_p = nc.declare_dram_parameter("cs", [256, SEQ], F32, isOutput=False)
    pat_p = nc.declare_dram_parameter("pat4", [KTILE, 4 * KTILE], F32, isOutput=False)
    id_p = nc.declare_dram_parameter("ident", [HD, HD], F16, isOutput=False)
    out_p = nc.declare_dram_parameter("out", [SEQ, DIM], F16, isOutput=True)

    xT_g = xT_p.rearrange("(k p) s -> p k s", p=128)
    wq_r = wq_p.rearrange("(k p) m -> p k m", p=128)
    wkv_r = wkv_p.rearrange("(k p) m -> p k m", p=128)
    EXP = mybir.ActivationFunctionType.Exp

    with TileContext(nc) as tc:
        with (
            tc.tile_pool(name="res", bufs=1) as res,
            tc.tile_pool(name="sb", bufs=2) as sb,
            tc.tile_pool(name="psum", bufs=1, space="PSUM") as psum,
        ):
            # ---- resident weights/constants ----
            wq_t = res.tile([128, NDCH, HPC * HD], F16, tag="wq_t")
            wkv_t = res.tile([128, NDCH, 2 * HD], F16, tag="wkv_t")
            wo0_t = res.tile([128, DIM], F16, tag="wo0_t")
            wo1_t = res.tile([128, DIM], F16, tag="wo1_t")
            cos4 = res.tile([128, SEQ], F32, tag="cos4")
            sin4 = res.tile([128, SEQ], F32, tag="sin4")
            pat4_t = res.tile([128, 4 * KTILE], F32, tag="pat4_t")
            ident = res.tile([HD, HD], F16, tag="ident")

            # ---- resident intermediates ----
            QeP = res.tile([128, SEQ], F16, tag="QeP")    # 4 heads x 32 even rows
            QoP = res.tile([128, SEQ], F16, tag="QoP")
            KrepE = res.tile([128, SEQ], F16, tag="KrepE")  # kv head x4 copies
            KrepO = res.tile([128, SEQ], F16, tag="KrepO")
            VT_sb = res.tile([HD, SEQ], F16, tag="VT_sb")
            OTn0 = res.tile([128, SEQ], F16, tag="OTn0")   # heads 0,1 normalized out^T
            OTn1 = res.tile([128, SEQ], F16, tag="OTn1")
            ones_col = res.tile([128, 1], F16, tag="ones_col")
            nc.vector.memset(ones_col[:], 1.0)
            ones_row = res.tile([1, HD], F16, tag="ones_row")
            nc.vector.memset(ones_row[:], 1.0)
            vext2 = []
            for kt in range(NKT):
                vx = res.tile([128, HD], F16, tag=f"vx{kt}", name=f"vx{kt}")
                vext2.append(vx)
            # denominator staging (base-partition-0 for the custom DVE op)
            den_in = [res.tile([1, SC], F32, tag=f"din{h}", name=f"din{h}")
                      for h in range(HPC)]
            rec = [res.tile([1, SC], F32, tag=f"rec{h}", name=f"rec{h}")
                   for h in range(HPC)]
            rec16 = [res.tile([1, SC], F16, tag=f"rec16_{h}", name=f"rec16_{h}")
                     for h in range(HPC)]

            ot4_box = [None]

            def op_tile(op_ps, j, src_sc, act_copy=True):
                """Out-projection tile j (0..15) of chunk src_sc: 2 matmuls +
                fp16 copy into a row-block buffer; one SWDGE DMA per seq row."""
                st = 4 * src_sc + j // 4
                dch = j % 4
                ssl = slice(st * 128, (st + 1) * 128)
                dsl = slice(dch * SC, (dch + 1) * SC)
                nc.tensor.matmul(op_ps[:], OTn0[:, ssl], wo0_t[:, dsl],
                                 start=True, stop=False)
                nc.tensor.matmul(op_ps[:], OTn1[:, ssl], wo1_t[:, dsl],
                                 start=False, stop=True)
                if dch == 0:
                    ot4_box[0] = sb.tile([128, DIM], F16, tag="ot", bufs=2, name=f"ot4_{src_sc}_{j}")
                nc.vector.tensor_copy(ot4_box[0][:, dsl], op_ps[:])
                if dch == 3:
                    nc.gpsimd.dma_start(out=out_p[ssl, :], in_=ot4_box[0][:])

            for _rep in range(reps):
                for sc in range(NSC):
                    slc = slice(sc * SC, (sc + 1) * SC)
                    # ---- proj(sc) + first half of outproj(sc-1) ----
                    projA = psum.tile([128, 2 * SC], F32, tag="st01", name=f"projA{sc}")
                    projB = psum.tile([128, SC], F32, tag="st23", name=f"projB{sc}")
                    qe_ps = projA[:, 0:SC]
                    qo_ps = projA[:, SC:2 * SC]
                    kv_ps = projB[:, 0:SC]
                    xt4 = None
                    for k in range(NDCH):
                        if sc == 0 and (_rep == 0 or reload_weights):
                            if k == 0:
                                nc.sync.dma_start(out=wq_t[:, 0:4, :], in_=wq_r[:, 0:4, :])
                                nc.sync.dma_start(out=wkv_t[:, 0:4, :], in_=wkv_r[:, 0:4, :])
                            if k == 1:
                                nc.sync.dma_start(out=wq_t[:, 4:8, :], in_=wq_r[:, 4:8, :])
                                nc.sync.dma_start(out=wkv_t[:, 4:8, :], in_=wkv_r[:, 4:8, :])
                            if k == 4:
                                nc.sync.dma_start(out=wq_t[:, 8:16, :], in_=wq_r[:, 8:16, :])
                                nc.sync.dma_start(out=wkv_t[:, 8:16, :], in_=wkv_r[:, 8:16, :])
                                nc.sync.dma_start(out=cos4[:], in_=cs_p[0:128, :])
                                nc.sync.dma_start(out=sin4[:], in_=cs_p[128:256, :])
                            if k == 8:
                                nc.sync.dma_start(out=pat4_t[:], in_=pat_p[:, :])
                                nc.sync.dma_start(out=ident[:], in_=id_p[:, :])
                                nc.sync.dma_start(out=wo0_t[:], in_=wo_p[0:128, :])
                                nc.sync.dma_start(out=wo1_t[:], in_=wo_p[128:256, :])
                        if k % 4 == 0:
                            xt4 = sb.tile([128, 4, SC], F16, tag="xt", bufs=3)
                            nc.sync.dma_start(out=xt4[:], in_=xT_g[:, k:k + 4, slc])
                        xt = xt4[:, k % 4, :]
                        st_, sp = (k == 0), (k == NDCH - 1)
                        nc.tensor.matmul(qe_ps[:], wq_t[:, k, 0:128], xt, start=st_, stop=sp)
                        nc.tensor.matmul(qo_ps[:], wq_t[:, k, 128:256], xt, start=st_, stop=sp)
                        nc.tensor.matmul(kv_ps[:], wkv_t[:, k, :], xt, start=st_, stop=sp)
                        if sc > 0 and k % 2 == 1:
                            op_ps = psum.tile([128, SC], F32, tag="op", name=f"op{sc}_{k}")
                            op_tile(op_ps, k // 2, sc - 1, act_copy=True)

                    # ---- rope(sc): Q full 128-lane, fp16 outputs ----
                    t1 = sb.tile([128, SC], F32, tag="t1", bufs=2)
                    t2 = sb.tile([128, SC], F32, tag="t2", bufs=2)
                    nc.vector.tensor_mul(t1[:], qe_ps[:], cos4[:, slc])
                    nc.vector.tensor_mul(t2[:], qo_ps[:], sin4[:, slc])
                    nc.vector.tensor_sub(QeP[:, slc], t1[:], t2[:])
                    t3 = sb.tile([128, SC], F32, tag="t3", bufs=2)
                    t4 = sb.tile([128, SC], F32, tag="t4", bufs=2)
                    nc.vector.tensor_mul(t3[:], qo_ps[:], cos4[:, slc])
                    nc.vector.tensor_mul(t4[:], qe_ps[:], sin4[:, slc])
                    nc.vector.tensor_add(QoP[:, slc], t3[:], t4[:])
                    # rope K into rows 0-31, then replicate x4 on GpSimd
                    k1 = sb.tile([32, SC], F32, tag="k1", bufs=2)
                    k2 = sb.tile([32, SC], F32, tag="k2", bufs=2)
                    nc.vector.tensor_mul(k1[:], kv_ps[0:32, :], cos4[0:32, slc])
                    nc.vector.tensor_mul(k2[:], kv_ps[32:64, :], sin4[0:32, slc])
                    nc.vector.tensor_sub(KrepE[0:32, slc], k1[:], k2[:])
                    k3 = sb.tile([32, SC], F32, tag="k3", bufs=2)
                    k4 = sb.tile([32, SC], F32, tag="k4", bufs=2)
                    nc.vector.tensor_mul(k3[:], kv_ps[32:64, :], cos4[0:32, slc])
                    nc.vector.tensor_mul(k4[:], kv_ps[0:32, :], sin4[0:32, slc])
                    nc.vector.tensor_add(KrepO[0:32, slc], k3[:], k4[:])
                    for r in (32, 64, 96):
                        nc.vector.tensor_copy(KrepE[r:r + 32, slc], KrepE[0:32, slc])
                        nc.vector.tensor_copy(KrepO[r:r + 32, slc], KrepO[0:32, slc])
                    # V passthrough (fp32 psum -> fp16)
                    nc.vector.tensor_copy(VT_sb[:, slc], kv_ps[HD:128, :])

                    # ---- vtrans(sc): V transpose, bank 7 ----
                    for kt in range(4 * sc, 4 * sc + 4):
                        vt_ps = psum.tile([128, HD], F16, tag="p7", name=f"vt{kt}")
                        nc.tensor.transpose(vt_ps[:], VT_sb[:, kt * 128:(kt + 1) * 128],
                                            ident[:])
                        nc.vector.tensor_copy(vext2[kt][:, 0:HD], vt_ps[:])

                    # ---- attention(sc) + second half of outproj(sc-1) ----
                    nkt_h = 4 * sc + 4
                    op_kts = [i * nkt_h // 8 for i in range(8)]
                    o2 = psum.tile([128, 2 * SC], F32, tag="o2", name=f"o2_{sc}")
                    o01 = o2[:, 0:SC]
                    o23 = o2[:, SC:2 * SC]
                    den4 = psum.tile([128, SC], F32, tag="p7", name=f"den{sc}")
                    nc.vector.memset(o2[:], 0.0)
                    nc.vector.memset(den4[:], 0.0)
                    for kt in range(nkt_h):
                        ksl = slice(kt * 128, (kt + 1) * 128)
                        j = kt - 4 * sc
                        qo = 128 * j if j > 0 else 0      # q offset within chunk
                        nv = SC - qo                      # valid q count
                        qsl = slice(sc * SC + qo, (sc + 1) * SC)
                        last = kt == nkt_h - 1
                        pt4 = sb.tile([128, 4 * SC], F16, tag="pt4", bufs=4)
                        for g in range(2):
                            stg = psum.tile([128, 2 * SC], F32, tag=f"st{'01' if g == 0 else '23'}",
                                            name=f"s{sc}_{kt}_{g}")
                            stv = stg[:].rearrange("p (b f) -> p b f", b=2)
                            for hh in range(2):
                                h = 2 * g + hh
                                rows = slice(32 * h, 32 * h + 32)
                                tp = (32 * h, 0)
                                nc.tensor.matmul(stg[:, hh * SC:hh * SC + nv],
                                                 KrepE[rows, ksl], QeP[rows, qsl],
                                                 start=True, stop=False, tile_position=tp)
                                nc.tensor.matmul(stg[:, hh * SC:hh * SC + nv],
                                                 KrepO[rows, ksl], QoP[rows, qsl],
                                                 start=False, stop=True, tile_position=tp)
                            if j >= 0:
                                nc.vector.tensor_add(stg[:, 0:128], stg[:, 0:128],
                                                     pat4_t[:, 0:128])
                                nc.vector.tensor_add(stg[:, SC:SC + 128], stg[:, SC:SC + 128],
                                                     pat4_t[:, 0:128])
                            ptv = pt4[:].rearrange("p (b f) -> p b f", b=4)[:, 2 * g:2 * g + 2, 0:nv]
                            nc.scalar.activation(ptv, stv[:, :, 0:nv], EXP, scale=0.125)
                            o_ps = (o01, o23)[g]
                            for hh in range(2):
                                h = 2 * g + hh
                                prhs = pt4[:, h * SC:h * SC + nv]
                                nc.tensor.matmul(o_ps[64 * hh:64 * hh + 64, qo:qo + nv],
                                                 vext2[kt][:, 0:HD], prhs,
                                                 start=False, stop=last,
                                                 tile_position=(0, 64 * hh),
                                                 skip_group_check=True)
                                nc.tensor.matmul(den4[32 * h:32 * h + 1, qo:qo + nv],
                                                 ones_col[:], prhs,
                                                 start=False, stop=last,
                                                 tile_position=(0, 32 * h),
                                                 skip_group_check=True)
                        if sc > 0 and kt in op_kts:
                            op_ps = psum.tile([128, SC], F32, tag="op", name=f"opa{sc}_{kt}")
                            op_tile(op_ps, 8 + op_kts.index(kt), sc - 1, act_copy=False)

                    # ---- norm(sc) ----
                    bc = psum.tile([128, 2 * SC], F32, tag="st01", name=f"bc{sc}")
                    for h in range(HPC):
                        nc.vector.tensor_copy(den_in[h][:], den4[32 * h:32 * h + 1, :])
                        nc.vector.reciprocal_approx_fast(rec[h][:], den_in[h][:])
                        nc.vector.tensor_copy(rec16[h][:], rec[h][:])
                    for g in range(2):
                        for hh in range(2):
                            h = 2 * g + hh
                            nc.tensor.matmul(bc[64 * hh:64 * hh + 64, g * SC:(g + 1) * SC],
                                             ones_row[:], rec16[h][:],
                                             start=True, stop=True,
                                             tile_position=(0, 64 * hh),
                                             skip_group_check=True)
                    bc_sb = sb.tile([128, 2 * SC], F16, tag="bc_sb", bufs=2)
                    nc.vector.tensor_copy(bc_sb[:, 0:SC], bc[:, 0:SC])
                    nc.vector.tensor_copy(bc_sb[:, SC:2 * SC], bc[:, SC:2 * SC])
                    nc.vector.tensor_mul(OTn0[:, slc], o01[:, :], bc_sb[:, 0:SC])
                    nc.vector.tensor_mul(OTn1[:, slc], o23[:, :], bc_sb[:, SC:2 * SC])

                # ---- tail outproj for last chunk (2-bank rotation) ----
                for i in range(NKT):
                    if i % 2 == 0:
                        op_ps = psum.tile([128, SC], F32, tag="op", name=f"opt{_rep}_{i}")
                    else:
                        op_ps = psum.tile([128, SC], F32, tag="st23", name=f"opt{_rep}_{i}")
                    op_tile(op_ps, i, NSC - 1, act_copy=(i % 2 == 0))

    nc.compile()
    return nc


def _host_prep(x, freqs_cos, freqs_sin):
    """Shared (core-independent) host-side tensors."""
    xT = np.ascontiguousarray(np.asarray(x, np.float32)[0].T).astype(np.float16)
    cosT = np.ascontiguousarray(np.asarray(freqs_cos, np.float32).T)   # [32, SEQ]
    sinT = np.ascontiguousarray(np.asarray(freqs_sin, np.float32).T)
    cs = np.concatenate([np.tile(cosT, (4, 1)), np.tile(sinT, (4, 1))], 0)  # [256, SEQ]
    kk = np.arange(KTILE)[:, None]
    qq = np.arange(KTILE)[None, :]
    pat = np.where(kk <= qq, 0.0, NEG).astype(np.float32)              # [128, 128]
    pat4 = np.ascontiguousarray(np.tile(pat, (1, 4)))                  # [128, 512]
    return xT, cs, pat4


def _perm_q():
    """wq columns -> [all heads' even dims (4x32), all heads' odd dims]."""
    ev = [h * HD + 2 * i for h in range(HPC) for i in range(HD // 2)]
    od = [h * HD + 2 * i + 1 for h in range(HPC) for i in range(HD // 2)]
    return ev + od


def _perm_k():
    """wk columns (single head) -> [even dims (32), odd dims (32)]."""
    return [2 * i for i in range(HD // 2)] + [2 * i + 1 for i in range(HD // 2)]


def _is_causal(mask):
    m = np.asarray(mask)
    if m.shape != (SEQ, SEQ):
        return False
    tril = np.tril(np.ones((SEQ, SEQ), bool))
    return bool(np.all(m[tril] == 0.0) and np.all(np.isneginf(m[~tril])))


def _numpy_fallback(x, freqs_cos, freqs_sin, mask, wq, wk, wv, wo):
    x = np.asarray(x, np.float64)
    b, s, _ = x.shape
    xq = (x @ wq).reshape(b, s, N_HEADS, HD)
    xk = (x @ wk).reshape(b, s, N_KV, HD)
    xv = (x @ wv).reshape(b, s, N_KV, HD)

    def rope(t):
        t2 = t.reshape(*t.shape[:-1], HD // 2, 2)
        te, to = t2[..., 0], t2[..., 1]
        c = np.asarray(freqs_cos, np.float64)[None, :, None, :]
        sn = np.asarray(freqs_sin, np.float64)[None, :, None, :]
        oe = te * c - to * sn
        oo = te * sn + to * c
        return np.stack([oe, oo], -1).reshape(t.shape)

    xq, xk = rope(xq), rope(xk)
    xk = np.repeat(xk, N_HEADS // N_KV, axis=2)
    xv = np.repeat(xv, N_HEADS // N_KV, axis=2)
    sc_ = np.einsum("bqhd,bkhd->bhqk", xq, xk) / np.sqrt(HD)
    sc_ = sc_ + np.asarray(mask, np.float64)[None, None]
    m = sc_.max(-1, keepdims=True)
    p = np.exp(sc_ - m)
    p = p / p.sum(-1, keepdims=True)
    out = np.einsum("bhqk,bkhd->bqhd", p, xv).reshape(b, s, N_HEADS * HD)
    return (out @ wo).astype(np.float32)


def _make_in_maps(x, freqs_cos, freqs_sin, wq, wk, wv, wo):
    xT, cs, pat4 = _host_prep(x, freqs_cos, freqs_sin)
    wq = np.asarray(wq, np.float32)
    wk = np.asarray(wk, np.float32)
    wv = np.asarray(wv, np.float32)
    wo = np.asarray(wo, np.float32)
    permq = _perm_q()
    permk = _perm_k()
    in_maps = []
    for c in range(NCORES):
        wq_c = np.ascontiguousarray(wq[:, c * 256:(c + 1) * 256][:, permq]).astype(np.float16)
        wk_c = wk[:, c * HD:(c + 1) * HD][:, permk]
        wv_c = wv[:, c * HD:(c + 1) * HD]
        wkv_c = np.ascontiguousarray(np.concatenate([wk_c, wv_c], 1)).astype(np.float16)
        wo_c = np.ascontiguousarray(wo[c * 256:(c + 1) * 256, :]).astype(np.float16)
        im = {"xT": xT, "wq": wq_c, "wkv": wkv_c, "wo": wo_c, "cs": cs, "pat4": pat4,
              "ident": np.eye(HD, dtype=np.float16)}
        in_maps.append(im)
    return in_maps


def get_nc(reps=1, reload_weights=True):
    key = f"nc{reps}_{reload_weights}"
    if key not in _CACHE:
        _CACHE[key] = _build_nc(reps, reload_weights)
    return _CACHE[key]


def kernel(x, freqs_cos, freqs_sin, mask, wq, wk, wv, wo):
    if not _is_causal(mask):
        return _numpy_fallback(x, freqs_cos, freqs_sin, mask, wq, wk, wv, wo)
    nc = get_nc()
    in_maps = _make_in_maps(x, freqs_cos, freqs_sin, wq, wk, wv, wo)
    res = run_bass_kernel_spmd(nc, in_maps, list(range(NCORES))).results
    acc = res[0]["out"].astype(np.float64)
    for c in range(1, NCORES):
        acc += res[c]["out"]
    return acc.astype(np.float32)[None]


# revision 3
# speedup vs baseline: 1.2045x; 1.0769x over previous
"""Trainium2 Bass kernel v2 for llama-style GQA attention block.

Per-core (8-way tensor parallel over heads): 4 q heads + 1 kv head, fp16
matmul operands everywhere (fp32 PSUM accumulation), fp16 partial output;
host sums the 8 partials.

Structure per seq chunk sc (SC=512):
  proj:   QKV projections (fp16 x/w) into PSUM pair-tiles st01 (banks 0-1:
          qe, qo) and st23 (bank 2: kv), with half of the PREVIOUS chunk's
          out-projection tiles interleaved into the k-loop.
  rope:   DVE fp32 math out of PSUM, fp16 outputs; K replication on GpSimd.
  vtrans: PE transposes V^T -> V (fp16 PSUM bank 7), duplicated into
          vext2=[V|V].
  attn:   per k-tile: 8 score matmuls (4 heads x even/odd, K=32 row-tiled at
          row groups 0/32/64/96) into st01/st23, per-pair fused exp on ACT
          (strided AP across 2 PSUM banks) -> fp16 pt4, col-tiled PV pairs
          into shared o-banks (memset + start=False accumulation), col-tiled
          M=1 denominator matmuls into bank-7 rows 0/32/64/96. Remaining
          out-projection tiles of chunk sc-1 interleave here.
  norm:   denominators -> base-0 tiles -> reciprocal_approx_fast -> PE
          broadcast (col-tiled pairs into st01) -> one fused TT per o-bank.
"""

import numpy as np

import concourse.bass as bass
import concourse.bacc as bacc
import concourse.mybir as mybir
from concourse.tile import TileContext
from concourse.bass_utils import run_bass_kernel_spmd

DIM = 2048
SEQ = 2048
N_HEADS = 32
N_KV = 8
HD = 64
NCORES = 8
HPC = N_HEADS // NCORES      # 4 q heads per core
SC = 512                     # seq chunk (matmul free dim)
NSC = SEQ // SC              # 4
KTILE = 128
NKT = SEQ // KTILE           # 16
NDCH = DIM // 128            # 16 contraction chunks for projections
F32 = mybir.dt.float32
F32R = mybir.dt.float32r
F16 = mybir.dt.float16
NEG = -1.0e30

_CACHE = {}


def _build_nc(reps=1, reload_weights=True):
    nc = bacc.Bacc("TRN2", debug=False, num_devices=NCORES)
    xT_p = nc.declare_dram_parameter("xT", [DIM, SEQ], F16, isOutput=False)
    wq_p = nc.declare_dram_parameter("wq", [DIM, HPC * HD], F16, isOutput=False)
    wkv_p = nc.declare_dram_parameter("wkv", [DIM, 2 * HD], F16, isOutput=False)
    wo_p = nc.declare_dram_parameter("wo", [HPC * HD, DIM], F16, isOutput=False)
    cs_p = nc.declare_dram_parameter("cs", [256, SEQ], F32, isOutput=False)
    pat_p = nc.declare_dram_parameter("pat4", [KTILE, 4 * KTILE], F32, isOutput=False)
    id_p = nc.declare_dram_parameter("ident", [HD, HD], F16, isOutput=False)
    out_p = nc.declare_dram_parameter("out", [SEQ, DIM], F16, isOutput=True)

    xT_g = xT_p.rearrange("(k p) s -> p k s", p=128)
    wq_r = wq_p.rearrange("(k p) m -> p k m", p=128)
    wkv_r = wkv_p.rearrange("(k p) m -> p k m", p=128)
    EXP = mybir.ActivationFunctionType.Exp

    with TileContext(nc) as tc:
        with (
            tc.tile_pool(name="res", bufs=1) as res,
            tc.tile_pool(name="sb", bufs=2) as sb,
            tc.tile_pool(name="psum", bufs=1, space="PSUM") as psum,
        ):
            # ---- resident weights/constants ----
            wq_t = res.tile([128, NDCH, HPC * HD], F16, tag="wq_t")
            wkv_t = res.tile([128, NDCH, 2 * HD], F16, tag="wkv_t")
            wo0_t = res.tile([128, DIM], F16, tag="wo0_t")
            wo1_t = res.tile([128, DIM], F16, tag="wo1_t")
            cos4 = res.tile([128, SEQ], F32, tag="cos4")
            sin4 = res.tile([128, SEQ], F32, tag="sin4")
            pat4_t = res.tile([128, 4 * KTILE], F32, tag="pat4_t")
            ident = res.tile([HD, HD], F16, tag="ident")

            # ---- resident intermediates ----
            QeP = res.tile([128, SEQ], F16, tag="QeP")    # 4 heads x 32 even rows
            QoP = res.tile([128, SEQ], F16, tag="QoP")
            KrepE = res.tile([128, SEQ], F16, tag="KrepE")  # kv head x4 copies
            KrepO = res.tile([128, SEQ], F16, tag="KrepO")
            VT_sb = res.tile([HD, SEQ], F16, tag="VT_sb")
            OTn0 = res.tile([128, SEQ], F16, tag="OTn0")   # heads 0,1 normalized out^T
            OTn1 = res.tile([128, SEQ], F16, tag="OTn1")
            ones_col = res.tile([128, 1], F16, tag="ones_col")
            nc.vector.memset(ones_col[:], 1.0)
            ones_row = res.tile([1, HD], F16, tag="ones_row")
            nc.vector.memset(ones_row[:], 1.0)
            vext2 = []
            for kt in range(NKT):
                vx = res.tile([128, HD], F16, tag=f"vx{kt}", name=f"vx{kt}")
                vext2.append(vx)
            # denominator staging (base-partition-0 for the custom DVE op)
            den_in = [res.tile([1, SC], F32, tag=f"din{h}", name=f"din{h}")
                      for h in range(HPC)]
            rec = [res.tile([1, SC], F32, tag=f"rec{h}", name=f"rec{h}")
                   for h in range(HPC)]
            rec16 = [res.tile([1, SC], F16, tag=f"rec16_{h}", name=f"rec16_{h}")
                     for h in range(HPC)]

            ot4_box = [None]

            def op_tile(op_ps, j, src_sc, act_copy=True):
                """Out-projection tile j (0..15) of chunk src_sc: 2 matmuls +
                fp16 copy into a row-block buffer; one SWDGE DMA per seq row."""
                st = 4 * src_sc + j // 4
                dch = j % 4
                ssl = slice(st * 128, (st + 1) * 128)
                dsl = slice(dch * SC, (dch + 1) * SC)
                nc.tensor.matmul(op_ps[:], OTn0[:, ssl], wo0_t[:, dsl],
                                 start=True, stop=False)
                nc.tensor.matmul(op_ps[:], OTn1[:, ssl], wo1_t[:, dsl],
                                 start=False, stop=True)
                if dch == 0:
                    ot4_box[0] = sb.tile([128, DIM], F16, tag="ot", bufs=2, name=f"ot4_{src_sc}_{j}")
                nc.vector.tensor_copy(ot4_box[0][:, dsl], op_ps[:])
                if dch == 3:
                    nc.gpsimd.dma_start(out=out_p[ssl, :], in_=ot4_box[0][:])

            for _rep in range(reps):
                pending_ops = []
                # ================= phase 1: proj + rope + vtrans, all chunks =================
                for sc in range(NSC):
                    slc = slice(sc * SC, (sc + 1) * SC)
                    if sc % 2 == 0:
                        projA = psum.tile([128, 2 * SC], F32, tag="st01", name=f"projA{_rep}_{sc}")
                        projB = psum.tile([128, SC], F32, tag="st23", name=f"projB{_rep}_{sc}")
                    else:
                        projA = psum.tile([128, 2 * SC], F32, tag="o2", name=f"projA{_rep}_{sc}")
                        projB = psum.tile([128, SC], F32, tag="op", name=f"projB{_rep}_{sc}")
                    qe_ps = projA[:, 0:SC]
                    qo_ps = projA[:, SC:2 * SC]
                    kv_ps = projB[:, 0:SC]
                    xt4 = None
                    for k in range(NDCH):
                        if sc == 0 and (_rep == 0 or reload_weights):
                            if k == 0:
                                nc.sync.dma_start(out=wq_t[:, 0:4, :], in_=wq_r[:, 0:4, :])
                                nc.sync.dma_start(out=wkv_t[:, 0:4, :], in_=wkv_r[:, 0:4, :])
                            if k == 1:
                                nc.sync.dma_start(out=wq_t[:, 4:8, :], in_=wq_r[:, 4:8, :])
                                nc.sync.dma_start(out=wkv_t[:, 4:8, :], in_=wkv_r[:, 4:8, :])
                            if k == 4:
                                nc.sync.dma_start(out=wq_t[:, 8:16, :], in_=wq_r[:, 8:16, :])
                                nc.sync.dma_start(out=wkv_t[:, 8:16, :], in_=wkv_r[:, 8:16, :])
                                nc.sync.dma_start(out=cos4[:], in_=cs_p[0:128, :])
                                nc.sync.dma_start(out=sin4[:], in_=cs_p[128:256, :])
                            if k == 8:
                                nc.sync.dma_start(out=pat4_t[:], in_=pat_p[:, :])
                                nc.sync.dma_start(out=ident[:], in_=id_p[:, :])
                                nc.sync.dma_start(out=wo0_t[:], in_=wo_p[0:128, :])
                                nc.sync.dma_start(out=wo1_t[:], in_=wo_p[128:256, :])
                        if k % 4 == 0:
                            xt4 = sb.tile([128, 4, SC], F16, tag="xt", bufs=3)
                            nc.sync.dma_start(out=xt4[:], in_=xT_g[:, k:k + 4, slc])
                        xt = xt4[:, k % 4, :]
                        st_, sp = (k == 0), (k == NDCH - 1)
                        nc.tensor.matmul(qe_ps[:], wq_t[:, k, 0:128], xt, start=st_, stop=sp)
                        nc.tensor.matmul(qo_ps[:], wq_t[:, k, 128:256], xt, start=st_, stop=sp)
                        nc.tensor.matmul(kv_ps[:], wkv_t[:, k, :], xt, start=st_, stop=sp)

                    # ---- rope(sc): Q full 128-lane, fp16 outputs ----
                    t1 = sb.tile([128, SC], F32, tag="t1", bufs=2)
                    t2 = sb.tile([128, SC], F32, tag="t2", bufs=2)
                    nc.vector.tensor_mul(t1[:], qe_ps[:], cos4[:, slc])
                    nc.vector.tensor_mul(t2[:], qo_ps[:], sin4[:, slc])
                    nc.vector.tensor_sub(QeP[:, slc], t1[:], t2[:])
                    t3 = sb.tile([128, SC], F32, tag="t3", bufs=2)
                    t4 = sb.tile([128, SC], F32, tag="t4", bufs=2)
                    nc.vector.tensor_mul(t3[:], qo_ps[:], cos4[:, slc])
                    nc.vector.tensor_mul(t4[:], qe_ps[:], sin4[:, slc])
                    nc.vector.tensor_add(QoP[:, slc], t3[:], t4[:])
                    # rope K into rows 0-31, then replicate x4 on GpSimd
                    k1 = sb.tile([32, SC], F32, tag="k1", bufs=2)
                    k2 = sb.tile([32, SC], F32, tag="k2", bufs=2)
                    nc.vector.tensor_mul(k1[:], kv_ps[0:32, :], cos4[0:32, slc])
                    nc.vector.tensor_mul(k2[:], kv_ps[32:64, :], sin4[0:32, slc])
                    nc.vector.tensor_sub(KrepE[0:32, slc], k1[:], k2[:])
                    k3 = sb.tile([32, SC], F32, tag="k3", bufs=2)
                    k4 = sb.tile([32, SC], F32, tag="k4", bufs=2)
                    nc.vector.tensor_mul(k3[:], kv_ps[32:64, :], cos4[0:32, slc])
                    nc.vector.tensor_mul(k4[:], kv_ps[0:32, :], sin4[0:32, slc])
                    nc.vector.tensor_add(KrepO[0:32, slc], k3[:], k4[:])
                    for r in (32, 64, 96):
                        nc.vector.tensor_copy(KrepE[r:r + 32, slc], KrepE[0:32, slc])
                        nc.vector.tensor_copy(KrepO[r:r + 32, slc], KrepO[0:32, slc])
                    # V passthrough (fp32 psum -> fp16)
                    nc.vector.tensor_copy(VT_sb[:, slc], kv_ps[HD:128, :])

                    # ---- vtrans(sc): V transpose, bank 7 ----
                    for kt in range(4 * sc, 4 * sc + 4):
                        vt_ps = psum.tile([128, HD], F16, tag="p7", name=f"vt{kt}")
                        nc.tensor.transpose(vt_ps[:], VT_sb[:, kt * 128:(kt + 1) * 128],
                                            ident[:])
                        nc.vector.tensor_copy(vext2[kt][:, 0:HD], vt_ps[:])

                # ================= phase 2: attention, all chunks =================
                for sc in range(NSC):
                    slc = slice(sc * SC, (sc + 1) * SC)
                    nkt_h = 4 * sc + 4
                    o2 = psum.tile([128, 2 * SC], F32, tag="o2", name=f"o2_{sc}")
                    o01 = o2[:, 0:SC]
                    o23 = o2[:, SC:2 * SC]
                    den4 = psum.tile([128, SC], F32, tag="p7", name=f"den{sc}")
                    nc.vector.memset(o2[:], 0.0)
                    nc.vector.memset(den4[:], 0.0)
                    for kt in range(nkt_h):
                        ksl = slice(kt * 128, (kt + 1) * 128)
                        j = kt - 4 * sc
                        qo = 128 * j if j > 0 else 0      # q offset within chunk
                        nv = SC - qo                      # valid q count
                        qsl = slice(sc * SC + qo, (sc + 1) * SC)
                        last = kt == nkt_h - 1
                        pt4 = sb.tile([128, 4 * SC], F16, tag="pt4", bufs=4)
                        for g in range(2):
                            stg = psum.tile([128, 2 * SC], F32, tag=f"st{'01' if g == 0 else '23'}",
                                            name=f"s{sc}_{kt}_{g}")
                            stv = stg[:].rearrange("p (b f) -> p b f", b=2)
                            for hh in range(2):
                                h = 2 * g + hh
                                rows = slice(32 * h, 32 * h + 32)
                                tp = (32 * h, 0)
                                nc.tensor.matmul(stg[:, hh * SC:hh * SC + nv],
                                                 KrepE[rows, ksl], QeP[rows, qsl],
                                                 start=True, stop=False, tile_position=tp)
                                nc.tensor.matmul(stg[:, hh * SC:hh * SC + nv],
                                                 KrepO[rows, ksl], QoP[rows, qsl],
                                                 start=False, stop=True, tile_position=tp)
                            if j >= 0:
                                nc.vector.tensor_add(stg[:, 0:128], stg[:, 0:128],
                                                     pat4_t[:, 0:128])
                                nc.vector.tensor_add(stg[:, SC:SC + 128], stg[:, SC:SC + 128],
                                                     pat4_t[:, 0:128])
                            ptv = pt4[:].rearrange("p (b f) -> p b f", b=4)[:, 2 * g:2 * g + 2, 0:nv]
                            nc.scalar.activation(ptv, stv[:, :, 0:nv], EXP, scale=0.125)
                            o_ps = (o01, o23)[g]
                            for hh in range(2):
                                h = 2 * g + hh
                                prhs = pt4[:, h * SC:h * SC + nv]
                                nc.tensor.matmul(o_ps[64 * hh:64 * hh + 64, qo:qo + nv],
                                                 vext2[kt][:, 0:HD], prhs,
                                                 start=False, stop=last,
                                                 tile_position=(0, 64 * hh),
                                                 skip_group_check=True)
                                nc.tensor.matmul(den4[32 * h:32 * h + 1, qo:qo + nv],
                                                 ones_col[:], prhs,
                                                 start=False, stop=last,
                                                 tile_position=(0, 32 * h),
                                                 skip_group_check=True)
                        for _f in range(2):
                            if pending_ops:
                                jj, ssc = pending_ops.pop(0)
                                op_ps = psum.tile([128, SC], F32, tag="op",
                                                  name=f"opa{_rep}_{sc}_{kt}_{_f}")
                                op_tile(op_ps, jj, ssc, act_copy=False)

                    # ---- norm(sc) ----
                    bc = psum.tile([128, 2 * SC], F32, tag="st01", name=f"bc{sc}")
                    for h in range(HPC):
                        nc.vector.tensor_copy(den_in[h][:], den4[32 * h:32 * h + 1, :])
                        nc.vector.reciprocal_approx_fast(rec[h][:], den_in[h][:])
                        nc.vector.tensor_copy(rec16[h][:], rec[h][:])
                    for g in range(2):
                        for hh in range(2):
                            h = 2 * g + hh
                            nc.tensor.matmul(bc[64 * hh:64 * hh + 64, g * SC:(g + 1) * SC],
                                             ones_row[:], rec16[h][:],
                                             start=True, stop=True,
                                             tile_position=(0, 64 * hh),
                                             skip_group_check=True)
                    bc_sb = sb.tile([128, 2 * SC], F16, tag="bc_sb", bufs=2)
                    nc.vector.tensor_copy(bc_sb[:, 0:SC], bc[:, 0:SC])
                    nc.vector.tensor_copy(bc_sb[:, SC:2 * SC], bc[:, SC:2 * SC])
                    nc.vector.tensor_mul(OTn0[:, slc], o01[:, :], bc_sb[:, 0:SC])
                    nc.vector.tensor_mul(OTn1[:, slc], o23[:, :], bc_sb[:, SC:2 * SC])
                    pending_ops.extend((j, sc) for j in range(NKT))

                # ---- tail outproj (2-bank rotation) ----
                for i, (jj, ssc) in enumerate(pending_ops):
                    if i % 2 == 0:
                        op_ps = psum.tile([128, SC], F32, tag="op", name=f"opt{_rep}_{i}")
                    else:
                        op_ps = psum.tile([128, SC], F32, tag="st23", name=f"opt{_rep}_{i}")
                    op_tile(op_ps, jj, ssc, act_copy=False)
                pending_ops = []

    nc.compile()
    return nc


def _host_prep(x, freqs_cos, freqs_sin):
    """Shared (core-independent) host-side tensors."""
    xT = np.ascontiguousarray(np.asarray(x, np.float32)[0].T).astype(np.float16)
    cosT = np.ascontiguousarray(np.asarray(freqs_cos, np.float32).T)   # [32, SEQ]
    sinT = np.ascontiguousarray(np.asarray(freqs_sin, np.float32).T)
    cs = np.concatenate([np.tile(cosT, (4, 1)), np.tile(sinT, (4, 1))], 0)  # [256, SEQ]
    kk = np.arange(KTILE)[:, None]
    qq = np.arange(KTILE)[None, :]
    pat = np.where(kk <= qq, 0.0, NEG).astype(np.float32)              # [128, 128]
    pat4 = np.ascontiguousarray(np.tile(pat, (1, 4)))                  # [128, 512]
    return xT, cs, pat4


def _perm_q():
    """wq columns -> [all heads' even dims (4x32), all heads' odd dims]."""
    ev = [h * HD + 2 * i for h in range(HPC) for i in range(HD // 2)]
    od = [h * HD + 2 * i + 1 for h in range(HPC) for i in range(HD // 2)]
    return ev + od


def _perm_k():
    """wk columns (single head) -> [even dims (32), odd dims (32)]."""
    return [2 * i for i in range(HD // 2)] + [2 * i + 1 for i in range(HD // 2)]


def _is_causal(mask):
    m = np.asarray(mask)
    if m.shape != (SEQ, SEQ):
        return False
    tril = np.tril(np.ones((SEQ, SEQ), bool))
    return bool(np.all(m[tril] == 0.0) and np.all(np.isneginf(m[~tril])))


def _numpy_fallback(x, freqs_cos, freqs_sin, mask, wq, wk, wv, wo):
    x = np.asarray(x, np.float64)
    b, s, _ = x.shape
    xq = (x @ wq).reshape(b, s, N_HEADS, HD)
    xk = (x @ wk).reshape(b, s, N_KV, HD)
    xv = (x @ wv).reshape(b, s, N_KV, HD)

    def rope(t):
        t2 = t.reshape(*t.shape[:-1], HD // 2, 2)
        te, to = t2[..., 0], t2[..., 1]
        c = np.asarray(freqs_cos, np.float64)[None, :, None, :]
        sn = np.asarray(freqs_sin, np.float64)[None, :, None, :]
        oe = te * c - to * sn
        oo = te * sn + to * c
        return np.stack([oe, oo], -1).reshape(t.shape)

    xq, xk = rope(xq), rope(xk)
    xk = np.repeat(xk, N_HEADS // N_KV, axis=2)
    xv = np.repeat(xv, N_HEADS // N_KV, axis=2)
    sc_ = np.einsum("bqhd,bkhd->bhqk", xq, xk) / np.sqrt(HD)
    sc_ = sc_ + np.asarray(mask, np.float64)[None, None]
    m = sc_.max(-1, keepdims=True)
    p = np.exp(sc_ - m)
    p = p / p.sum(-1, keepdims=True)
    out = np.einsum("bhqk,bkhd->bqhd", p, xv).reshape(b, s, N_HEADS * HD)
    return (out @ wo).astype(np.float32)


def _make_in_maps(x, freqs_cos, freqs_sin, wq, wk, wv, wo):
    xT, cs, pat4 = _host_prep(x, freqs_cos, freqs_sin)
    wq = np.asarray(wq, np.float32)
    wk = np.asarray(wk, np.float32)
    wv = np.asarray(wv, np.float32)
    wo = np.asarray(wo, np.float32)
    permq = _perm_q()
    permk = _perm_k()
    in_maps = []
    for c in range(NCORES):
        wq_c = np.ascontiguousarray(wq[:, c * 256:(c + 1) * 256][:, permq]).astype(np.float16)
        wk_c = wk[:, c * HD:(c + 1) * HD][:, permk]
        wv_c = wv[:, c * HD:(c + 1) * HD]
        wkv_c = np.ascontiguousarray(np.concatenate([wk_c, wv_c], 1)).astype(np.float16)
        wo_c = np.ascontiguousarray(wo[c * 256:(c + 1) * 256, :]).astype(np.float16)
        im = {"xT": xT, "wq": wq_c, "wkv": wkv_c, "wo": wo_c, "cs": cs, "pat4": pat4,
              "ident": np.eye(HD, dtype=np.float16)}
        in_maps.append(im)
    return in_maps


def get_nc(reps=1, reload_weights=True):
    key = f"nc{reps}_{reload_weights}"
    if key not in _CACHE:
        _CACHE[key] = _build_nc(reps, reload_weights)
    return _CACHE[key]


def kernel(x, freqs_cos, freqs_sin, mask, wq, wk, wv, wo):
    if not _is_causal(mask):
        return _numpy_fallback(x, freqs_cos, freqs_sin, mask, wq, wk, wv, wo)
    nc = get_nc()
    in_maps = _make_in_maps(x, freqs_cos, freqs_sin, wq, wk, wv, wo)
    res = run_bass_kernel_spmd(nc, in_maps, list(range(NCORES))).results
    acc = res[0]["out"].astype(np.float64)
    for c in range(1, NCORES):
        acc += res[c]["out"]
    return acc.astype(np.float32)[None]


# revision 4
# speedup vs baseline: 1.5514x; 1.2880x over previous
"""Trainium2 Bass kernel v2 for llama-style GQA attention block.

Per-core (8-way tensor parallel over heads): 4 q heads + 1 kv head, fp16
matmul operands everywhere (fp32 PSUM accumulation), fp16 partial output;
host sums the 8 partials.

Structure per seq chunk sc (SC=512):
  proj:   QKV projections (fp16 x/w) into PSUM pair-tiles st01 (banks 0-1:
          qe, qo) and st23 (bank 2: kv), with half of the PREVIOUS chunk's
          out-projection tiles interleaved into the k-loop.
  rope:   DVE fp32 math out of PSUM, fp16 outputs; K replication on GpSimd.
  vtrans: PE transposes V^T -> V (fp16 PSUM bank 7), duplicated into
          vext2=[V|V].
  attn:   per k-tile: 8 score matmuls (4 heads x even/odd, K=32 row-tiled at
          row groups 0/32/64/96) into st01/st23, per-pair fused exp on ACT
          (strided AP across 2 PSUM banks) -> fp16 pt4, col-tiled PV pairs
          into shared o-banks (memset + start=False accumulation), col-tiled
          M=1 denominator matmuls into bank-7 rows 0/32/64/96. Remaining
          out-projection tiles of chunk sc-1 interleave here.
  norm:   denominators -> base-0 tiles -> reciprocal_approx_fast -> PE
          broadcast (col-tiled pairs into st01) -> one fused TT per o-bank.
"""

import numpy as np

import concourse.bass as bass
import concourse.bacc as bacc
import concourse.mybir as mybir
from concourse.tile import TileContext
from concourse.bass_utils import run_bass_kernel_spmd

DIM = 2048
SEQ = 2048
N_HEADS = 32
N_KV = 8
HD = 64
NCORES = 8
HPC = N_HEADS // NCORES      # 4 q heads per core
SC = 512                     # seq chunk (matmul free dim)
NSC = SEQ // SC              # 4
KTILE = 128
NKT = SEQ // KTILE           # 16
NDCH = DIM // 128            # 16 contraction chunks for projections
F32 = mybir.dt.float32
F32R = mybir.dt.float32r
F16 = mybir.dt.float16
NEG = -1.0e30

_CACHE = {}


def _build_nc(reps=1, reload_weights=True):
    nc = bacc.Bacc("TRN2", debug=False, num_devices=NCORES)
    xT_p = nc.declare_dram_parameter("xT", [DIM, SEQ], F16, isOutput=False)
    wq_p = nc.declare_dram_parameter("wq", [DIM, HPC * HD], F16, isOutput=False)
    wkv_p = nc.declare_dram_parameter("wkv", [DIM, 2 * HD], F16, isOutput=False)
    wo_p = nc.declare_dram_parameter("wo", [HPC * HD, DIM], F16, isOutput=False)
    cs_p = nc.declare_dram_parameter("cs", [256, SEQ], F32, isOutput=False)
    pat_p = nc.declare_dram_parameter("pat4", [KTILE, 4 * KTILE], F32, isOutput=False)
    id_p = nc.declare_dram_parameter("ident", [HD, HD], F16, isOutput=False)
    out_p = nc.declare_dram_parameter("out", [SEQ, DIM], F16, isOutput=True)

    xT_g = xT_p.rearrange("(k p) s -> p k s", p=128)
    wq_r = wq_p.rearrange("(k p) m -> p k m", p=128)
    wkv_r = wkv_p.rearrange("(k p) m -> p k m", p=128)
    EXP = mybir.ActivationFunctionType.Exp

    with TileContext(nc) as tc:
        with (
            tc.tile_pool(name="res", bufs=1) as res,
            tc.tile_pool(name="sb", bufs=2) as sb,
            tc.tile_pool(name="psum", bufs=1, space="PSUM") as psum,
        ):
            # ---- resident weights/constants ----
            wq_t = res.tile([128, NDCH, HPC * HD], F16, tag="wq_t")
            wkv_t = res.tile([128, NDCH, 2 * HD], F16, tag="wkv_t")
            wo0_t = res.tile([128, DIM], F16, tag="wo0_t")
            wo1_t = res.tile([128, DIM], F16, tag="wo1_t")
            cos4 = res.tile([128, SEQ], F32, tag="cos4")
            sin4 = res.tile([128, SEQ], F32, tag="sin4")
            pat4_t = res.tile([128, 4 * KTILE], F32, tag="pat4_t")
            ident = res.tile([HD, HD], F16, tag="ident")

            # ---- resident intermediates ----
            QeP = res.tile([128, SEQ], F16, tag="QeP")    # 4 heads x 32 even rows
            QoP = res.tile([128, SEQ], F16, tag="QoP")
            KrepE = res.tile([128, SEQ], F16, tag="KrepE")  # kv head x4 copies
            KrepO = res.tile([128, SEQ], F16, tag="KrepO")
            VT_sb = res.tile([HD, SEQ], F16, tag="VT_sb")
            OTn0 = res.tile([128, SEQ], F16, tag="OTn0")   # heads 0,1 normalized out^T
            OTn1 = res.tile([128, SEQ], F16, tag="OTn1")
            ones_col = res.tile([128, 1], F16, tag="ones_col")
            nc.vector.memset(ones_col[:], 1.0)
            ones_row = res.tile([1, HD], F16, tag="ones_row")
            nc.vector.memset(ones_row[:], 1.0)
            tri01 = res.tile([128, KTILE], F16, tag="tri01")
            nbias = res.tile([128, 1], F32, tag="nbias")
            nc.vector.memset(nbias[:], -2.0)
            vext2 = []
            for kt in range(NKT):
                vx = res.tile([128, HD], F16, tag=f"vx{kt}", name=f"vx{kt}")
                vext2.append(vx)
            # denominator staging (base-partition-0 for the custom DVE op)
            den_in = [res.tile([1, SC], F32, tag=f"din{h}", name=f"din{h}")
                      for h in range(HPC)]
            rec = [res.tile([1, SC], F32, tag=f"rec{h}", name=f"rec{h}")
                   for h in range(HPC)]
            rec16 = [res.tile([1, SC], F16, tag=f"rec16_{h}", name=f"rec16_{h}")
                     for h in range(HPC)]

            ot4_box = [None]

            def op_tile(op_ps, j, src_sc, act_copy=True):
                """Out-projection tile j (0..15) of chunk src_sc: 2 matmuls +
                fp16 copy into a row-block buffer; one SWDGE DMA per seq row."""
                st = 4 * src_sc + j // 4
                dch = j % 4
                ssl = slice(st * 128, (st + 1) * 128)
                dsl = slice(dch * SC, (dch + 1) * SC)
                nc.tensor.matmul(op_ps[:], OTn0[:, ssl], wo0_t[:, dsl],
                                 start=True, stop=False)
                nc.tensor.matmul(op_ps[:], OTn1[:, ssl], wo1_t[:, dsl],
                                 start=False, stop=True)
                if dch == 0:
                    ot4_box[0] = sb.tile([128, DIM], F16, tag="ot", bufs=2, name=f"ot4_{src_sc}_{j}")
                nc.vector.tensor_copy(ot4_box[0][:, dsl], op_ps[:])
                if dch == 3:
                    nc.gpsimd.dma_start(out=out_p[ssl, :], in_=ot4_box[0][:])

            tri_done = [False]

            for _rep in range(reps):
                pending_ops = []
                # ================= phase 1: proj + rope + vtrans, all chunks =================
                for sc in range(NSC):
                    slc = slice(sc * SC, (sc + 1) * SC)
                    if sc % 2 == 0:
                        projA = psum.tile([128, 2 * SC], F32, tag="st01", name=f"projA{_rep}_{sc}")
                        projB = psum.tile([128, SC], F32, tag="st23", name=f"projB{_rep}_{sc}")
                    else:
                        projA = psum.tile([128, 2 * SC], F32, tag="o2", name=f"projA{_rep}_{sc}")
                        projB = psum.tile([128, SC], F32, tag="op", name=f"projB{_rep}_{sc}")
                    qe_ps = projA[:, 0:SC]
                    qo_ps = projA[:, SC:2 * SC]
                    kv_ps = projB[:, 0:SC]
                    xt4 = None
                    for k in range(NDCH):
                        if sc == 0 and (_rep == 0 or reload_weights):
                            if k == 0:
                                nc.sync.dma_start(out=wq_t[:, 0:4, :], in_=wq_r[:, 0:4, :])
                                nc.sync.dma_start(out=wkv_t[:, 0:4, :], in_=wkv_r[:, 0:4, :])
                            if k == 1:
                                nc.sync.dma_start(out=wq_t[:, 4:8, :], in_=wq_r[:, 4:8, :])
                                nc.sync.dma_start(out=wkv_t[:, 4:8, :], in_=wkv_r[:, 4:8, :])
                            if k == 4:
                                nc.sync.dma_start(out=wq_t[:, 8:16, :], in_=wq_r[:, 8:16, :])
                                nc.sync.dma_start(out=wkv_t[:, 8:16, :], in_=wkv_r[:, 8:16, :])
                                nc.sync.dma_start(out=cos4[:], in_=cs_p[0:128, :])
                                nc.sync.dma_start(out=sin4[:], in_=cs_p[128:256, :])
                            if k == 8:
                                nc.sync.dma_start(out=pat4_t[:], in_=pat_p[:, :])
                                nc.sync.dma_start(out=ident[:], in_=id_p[:, :])
                                nc.sync.dma_start(out=wo0_t[:], in_=wo_p[0:128, :])
                                nc.sync.dma_start(out=wo1_t[:], in_=wo_p[128:256, :])
                        if k % 4 == 0:
                            xt4 = sb.tile([128, 4, SC], F16, tag="xt", bufs=3)
                            eng = nc.sync if (k // 4) % 2 == 0 else nc.scalar
                            eng.dma_start(out=xt4[:], in_=xT_g[:, k:k + 4, slc])
                        xt = xt4[:, k % 4, :]
                        st_, sp = (k == 0), (k == NDCH - 1)
                        nc.tensor.matmul(qe_ps[:], wq_t[:, k, 0:128], xt, start=st_, stop=sp)
                        nc.tensor.matmul(qo_ps[:], wq_t[:, k, 128:256], xt, start=st_, stop=sp)
                        nc.tensor.matmul(kv_ps[:], wkv_t[:, k, :], xt, start=st_, stop=sp)

                    # ---- V passthrough + vtrans first (short DVE chain for PE) ----
                    nc.vector.tensor_copy(VT_sb[:, slc], kv_ps[HD:128, :])
                    for kt in range(4 * sc, 4 * sc + 4):
                        vt_ps = psum.tile([128, HD], F16, tag="p7", name=f"vt{_rep}_{kt}")
                        nc.tensor.transpose(vt_ps[:], VT_sb[:, kt * 128:(kt + 1) * 128],
                                            ident[:])
                        nc.vector.tensor_copy(vext2[kt][:, 0:HD], vt_ps[:])

                    # ---- rope(sc): Q full 128-lane, fp16 outputs ----
                    t1 = sb.tile([128, SC], F32, tag="t1", bufs=2)
                    t2 = sb.tile([128, SC], F32, tag="t2", bufs=2)
                    nc.vector.tensor_mul(t1[:], qe_ps[:], cos4[:, slc])
                    nc.vector.tensor_mul(t2[:], qo_ps[:], sin4[:, slc])
                    nc.vector.tensor_sub(QeP[:, slc], t1[:], t2[:])
                    t3 = sb.tile([128, SC], F32, tag="t3", bufs=2)
                    t4 = sb.tile([128, SC], F32, tag="t4", bufs=2)
                    nc.vector.tensor_mul(t3[:], qo_ps[:], cos4[:, slc])
                    nc.vector.tensor_mul(t4[:], qe_ps[:], sin4[:, slc])
                    nc.vector.tensor_add(QoP[:, slc], t3[:], t4[:])
                    # rope K into rows 0-31, then replicate x4 on GpSimd
                    k1 = sb.tile([32, SC], F32, tag="k1", bufs=2)
                    k2 = sb.tile([32, SC], F32, tag="k2", bufs=2)
                    nc.vector.tensor_mul(k1[:], kv_ps[0:32, :], cos4[0:32, slc])
                    nc.vector.tensor_mul(k2[:], kv_ps[32:64, :], sin4[0:32, slc])
                    nc.vector.tensor_sub(KrepE[0:32, slc], k1[:], k2[:])
                    k3 = sb.tile([32, SC], F32, tag="k3", bufs=2)
                    k4 = sb.tile([32, SC], F32, tag="k4", bufs=2)
                    nc.vector.tensor_mul(k3[:], kv_ps[32:64, :], cos4[0:32, slc])
                    nc.vector.tensor_mul(k4[:], kv_ps[0:32, :], sin4[0:32, slc])
                    nc.vector.tensor_add(KrepO[0:32, slc], k3[:], k4[:])
                    for r in (32, 64, 96):
                        nc.vector.tensor_copy(KrepE[r:r + 32, slc], KrepE[0:32, slc])
                        nc.vector.tensor_copy(KrepO[r:r + 32, slc], KrepO[0:32, slc])
                # ================= phase 2: attention, all chunks =================
                for sc in range(NSC):
                    slc = slice(sc * SC, (sc + 1) * SC)
                    nkt_h = 4 * sc + 4
                    o2 = psum.tile([128, 2 * SC], F32, tag="o2", name=f"o2_{sc}")
                    o01 = o2[:, 0:SC]
                    o23 = o2[:, SC:2 * SC]
                    den4 = psum.tile([128, SC], F32, tag="p7", name=f"den{sc}")
                    nc.vector.memset(o2[:], 0.0)
                    nc.vector.memset(den4[:], 0.0)
                    for kt in range(nkt_h):
                        ksl = slice(kt * 128, (kt + 1) * 128)
                        j = kt - 4 * sc
                        qo = 128 * j if j > 0 else 0      # q offset within chunk
                        nv = SC - qo                      # valid q count
                        qsl = slice(sc * SC + qo, (sc + 1) * SC)
                        last = kt == nkt_h - 1
                        pt4 = sb.tile([128, 4 * SC], F16, tag="pt4", bufs=4)
                        for g in range(2):
                            stg = psum.tile([128, 2 * SC], F32, tag=f"st{'01' if g == 0 else '23'}",
                                            name=f"s{sc}_{kt}_{g}")
                            stv = stg[:].rearrange("p (b f) -> p b f", b=2)
                            for hh in range(2):
                                h = 2 * g + hh
                                rows = slice(32 * h, 32 * h + 32)
                                tp = (32 * h, 0)
                                nc.tensor.matmul(stg[:, hh * SC:hh * SC + nv],
                                                 KrepE[rows, ksl], QeP[rows, qsl],
                                                 start=True, stop=False, tile_position=tp)
                                nc.tensor.matmul(stg[:, hh * SC:hh * SC + nv],
                                                 KrepO[rows, ksl], QoP[rows, qsl],
                                                 start=False, stop=True, tile_position=tp)
                            ptv = pt4[:].rearrange("p (b f) -> p b f", b=4)[:, 2 * g:2 * g + 2, 0:nv]
                            nc.scalar.activation(ptv, stv[:, :, 0:nv], EXP, scale=0.125, bias=nbias[:])
                            if j >= 0:
                                if not tri_done[0]:
                                    nc.scalar.activation(tri01[:], pat4_t[:, 0:128], EXP)
                                    tri_done[0] = True
                                for hh in range(2):
                                    h = 2 * g + hh
                                    nc.vector.tensor_mul(pt4[:, h * SC:h * SC + 128],
                                                         pt4[:, h * SC:h * SC + 128],
                                                         tri01[:])
                            o_ps = (o01, o23)[g]
                            for hh in range(2):
                                h = 2 * g + hh
                                prhs = pt4[:, h * SC:h * SC + nv]
                                nc.tensor.matmul(o_ps[64 * hh:64 * hh + 64, qo:qo + nv],
                                                 vext2[kt][:, 0:HD], prhs,
                                                 start=False, stop=last,
                                                 tile_position=(0, 64 * hh),
                                                 skip_group_check=True)
                                nc.tensor.matmul(den4[32 * h:32 * h + 1, qo:qo + nv],
                                                 ones_col[:], prhs,
                                                 start=False, stop=last,
                                                 tile_position=(0, 32 * h),
                                                 skip_group_check=True)
                        for _f in range(2):
                            if pending_ops:
                                jj, ssc = pending_ops.pop(0)
                                op_ps = psum.tile([128, SC], F32, tag="op",
                                                  name=f"opa{_rep}_{sc}_{kt}_{_f}")
                                op_tile(op_ps, jj, ssc, act_copy=False)

                    # ---- norm(sc): bc on the den bank (p7) so st01/st23 stay
                    # free for the next chunk's score conveyor ----
                    for h in range(HPC):
                        nc.vector.tensor_copy(den_in[h][:], den4[32 * h:32 * h + 1, :])
                        nc.vector.reciprocal_approx_fast(rec[h][:], den_in[h][:])
                        nc.vector.tensor_copy(rec16[h][:], rec[h][:])
                    bc_sb = sb.tile([128, 2 * SC], F16, tag="bc_sb", bufs=2)
                    for g in range(2):
                        bcg = psum.tile([128, SC], F32, tag="p7", name=f"bc{_rep}_{sc}_{g}")
                        for hh in range(2):
                            h = 2 * g + hh
                            nc.tensor.matmul(bcg[64 * hh:64 * hh + 64, :],
                                             ones_row[:], rec16[h][:],
                                             start=True, stop=True,
                                             tile_position=(0, 64 * hh),
                                             skip_group_check=True)
                        nc.vector.tensor_copy(bc_sb[:, g * SC:(g + 1) * SC], bcg[:])
                    nc.vector.tensor_mul(OTn0[:, slc], o01[:, :], bc_sb[:, 0:SC])
                    nc.vector.tensor_mul(OTn1[:, slc], o23[:, :], bc_sb[:, SC:2 * SC])
                    pending_ops.extend((j, sc) for j in range(NKT))

                # ---- tail outproj (2-bank rotation) ----
                for i, (jj, ssc) in enumerate(pending_ops):
                    if i % 2 == 0:
                        op_ps = psum.tile([128, SC], F32, tag="op", name=f"opt{_rep}_{i}")
                    else:
                        op_ps = psum.tile([128, SC], F32, tag="st23", name=f"opt{_rep}_{i}")
                    op_tile(op_ps, jj, ssc, act_copy=False)
                pending_ops = []

    nc.compile()
    return nc


def _host_prep(x, freqs_cos, freqs_sin):
    """Shared (core-independent) host-side tensors."""
    xT = np.ascontiguousarray(np.asarray(x, np.float32)[0].T).astype(np.float16)
    cosT = np.ascontiguousarray(np.asarray(freqs_cos, np.float32).T)   # [32, SEQ]
    sinT = np.ascontiguousarray(np.asarray(freqs_sin, np.float32).T)
    cs = np.concatenate([np.tile(cosT, (4, 1)), np.tile(sinT, (4, 1))], 0)  # [256, SEQ]
    kk = np.arange(KTILE)[:, None]
    qq = np.arange(KTILE)[None, :]
    pat = np.where(kk <= qq, 0.0, NEG).astype(np.float32)              # [128, 128]
    pat4 = np.ascontiguousarray(np.tile(pat, (1, 4)))                  # [128, 512]
    return xT, cs, pat4


def _perm_q():
    """wq columns -> [all heads' even dims (4x32), all heads' odd dims]."""
    ev = [h * HD + 2 * i for h in range(HPC) for i in range(HD // 2)]
    od = [h * HD + 2 * i + 1 for h in range(HPC) for i in range(HD // 2)]
    return ev + od


def _perm_k():
    """wk columns (single head) -> [even dims (32), odd dims (32)]."""
    return [2 * i for i in range(HD // 2)] + [2 * i + 1 for i in range(HD // 2)]


def _is_causal(mask):
    m = np.asarray(mask)
    if m.shape != (SEQ, SEQ):
        return False
    tril = np.tril(np.ones((SEQ, SEQ), bool))
    return bool(np.all(m[tril] == 0.0) and np.all(np.isneginf(m[~tril])))


def _numpy_fallback(x, freqs_cos, freqs_sin, mask, wq, wk, wv, wo):
    x = np.asarray(x, np.float64)
    b, s, _ = x.shape
    xq = (x @ wq).reshape(b, s, N_HEADS, HD)
    xk = (x @ wk).reshape(b, s, N_KV, HD)
    xv = (x @ wv).reshape(b, s, N_KV, HD)

    def rope(t):
        t2 = t.reshape(*t.shape[:-1], HD // 2, 2)
        te, to = t2[..., 0], t2[..., 1]
        c = np.asarray(freqs_cos, np.float64)[None, :, None, :]
        sn = np.asarray(freqs_sin, np.float64)[None, :, None, :]
        oe = te * c - to * sn
        oo = te * sn + to * c
        return np.stack([oe, oo], -1).reshape(t.shape)

    xq, xk = rope(xq), rope(xk)
    xk = np.repeat(xk, N_HEADS // N_KV, axis=2)
    xv = np.repeat(xv, N_HEADS // N_KV, axis=2)
    sc_ = np.einsum("bqhd,bkhd->bhqk", xq, xk) / np.sqrt(HD)
    sc_ = sc_ + np.asarray(mask, np.float64)[None, None]
    m = sc_.max(-1, keepdims=True)
    p = np.exp(sc_ - m)
    p = p / p.sum(-1, keepdims=True)
    out = np.einsum("bhqk,bkhd->bqhd", p, xv).reshape(b, s, N_HEADS * HD)
    return (out @ wo).astype(np.float32)


def _make_in_maps(x, freqs_cos, freqs_sin, wq, wk, wv, wo):
    xT, cs, pat4 = _host_prep(x, freqs_cos, freqs_sin)
    wq = np.asarray(wq, np.float32)
    wk = np.asarray(wk, np.float32)
    wv = np.asarray(wv, np.float32)
    wo = np.asarray(wo, np.float32)
    permq = _perm_q()
    permk = _perm_k()
    in_maps = []
    for c in range(NCORES):
        wq_c = np.ascontiguousarray(wq[:, c * 256:(c + 1) * 256][:, permq]).astype(np.float16)
        wk_c = wk[:, c * HD:(c + 1) * HD][:, permk]
        wv_c = wv[:, c * HD:(c + 1) * HD]
        wkv_c = np.ascontiguousarray(np.concatenate([wk_c, wv_c], 1)).astype(np.float16)
        wo_c = np.ascontiguousarray(wo[c * 256:(c + 1) * 256, :]).astype(np.float16)
        im = {"xT": xT, "wq": wq_c, "wkv": wkv_c, "wo": wo_c, "cs": cs, "pat4": pat4,
              "ident": np.eye(HD, dtype=np.float16)}
        in_maps.append(im)
    return in_maps


def get_nc(reps=1, reload_weights=True):
    key = f"nc{reps}_{reload_weights}"
    if key not in _CACHE:
        _CACHE[key] = _build_nc(reps, reload_weights)
    return _CACHE[key]


def kernel(x, freqs_cos, freqs_sin, mask, wq, wk, wv, wo):
    if not _is_causal(mask):
        return _numpy_fallback(x, freqs_cos, freqs_sin, mask, wq, wk, wv, wo)
    nc = get_nc()
    in_maps = _make_in_maps(x, freqs_cos, freqs_sin, wq, wk, wv, wo)
    res = run_bass_kernel_spmd(nc, in_maps, list(range(NCORES))).results
    acc = res[0]["out"].astype(np.float64)
    for c in range(1, NCORES):
        acc += res[c]["out"]
    return acc.astype(np.float32)[None]
